# revision 20
# baseline (speedup 1.0000x reference)
import math
import sys

sys.path.insert(0, "/opt/trn_rl_repo")

import numpy as np

# ---- model constants (from the reference nn.Module) ----
ROPE_PERIOD = 19.0
OMEGA = 2.0 * math.pi / ROPE_PERIOD
PEAK_EPS = 0.3
TARGET_LOGIT_GAP = math.log(10.0)
ATTN_AMPLITUDE = TARGET_LOGIT_GAP / (
    math.cos(OMEGA * PEAK_EPS) - math.cos(OMEGA * (1.0 - PEAK_EPS))
)
QK_NORM_SCALE = math.sqrt(ATTN_AMPLITUDE / math.sqrt(2.0))
SCALE = 2.0 ** (-0.5) * QK_NORM_SCALE**2
EMBED_CONST = 1000.0
EPS = 1e-6

B, L = 4, 4096
N_CORES = 8
NCH = 16            # owned 256-wide chunk windows per core
SEGW = 2048         # psum segment width (4 banks)
NEG = -10240.0      # additive causal mask value (exact in bf16)
SQ2 = math.sqrt(2.0)

_compiled = None


def _build():
    import concourse.bass as bass  # noqa: F401
    import concourse.tile as tile
    from concourse import bacc, mybir

    f32 = mybir.dt.float32
    bf16 = mybir.dt.bfloat16
    AF = mybir.ActivationFunctionType
    OP = mybir.AluOpType

    from concourse.hw_specs import get_activation_tables

    nc = bacc.Bacc("TRN2", target_bir_lowering=False, debug=False,
                   num_devices=N_CORES)
    _tab_names = list(get_activation_tables(nc.m.arch).keys())
    LNEXP_SET = _tab_names.index("natural_log_exp_and_others")

    xin_d = nc.dram_tensor("xin", [128, 224], f32, kind="ExternalInput").ap()
    wts_d = nc.dram_tensor("wts", [128, 8], f32, kind="ExternalInput").ap()
    msk_d = nc.dram_tensor("msk", [128, 384], bf16, kind="ExternalInput").ap()
    out_d = nc.dram_tensor("out", [128, 32], f32, kind="ExternalOutput").ap()

    with tile.TileContext(nc) as tc:
        with (
            tc.tile_pool(name="const", bufs=1) as cp,
            tc.tile_pool(name="work", bufs=2) as wp,
            tc.tile_pool(name="ep", bufs=3) as ep,
            tc.tile_pool(name="pmm", bufs=2, space="PSUM") as pmm,
            tc.tile_pool(name="dram", bufs=1, space="DRAM") as dp,
        ):
            # preload the combined ln+exp activation table set
            nc.scalar.add_instruction(mybir.InstLoadActFuncSet(
                name=nc.get_next_instruction_name(),
                act_func_set_id=LNEXP_SET, ins=[], outs=[]))

            # ---------- input DMAs: 3 combined loads ----------
            wts = cp.tile([128, 8], f32, tag="wts")
            nc.sync.dma_start(wts[:], wts_d[:])
            xin = cp.tile([128, 224], f32, tag="xin")
            nc.gpsimd.dma_start(xin[:], xin_d[:])
            msk = cp.tile([128, 384], bf16, tag="msk")
            nc.sync.dma_start(msk[:], msk_d[:])
            wq, wv = wts[:, 0:1], wts[:, 1:2]
            wga, wgc, wc = wts[:, 2:3], wts[:, 3:4], wts[:, 4:5]
            xq0, xq1 = xin[:, 0:32], xin[:, 32:64]
            cth, sth = xin[:, 64:96], xin[:, 96:128]
            xqo0, xqo1 = xin[:, 128:144], xin[:, 144:160]
            ctho, stho = xin[:, 160:176], xin[:, 176:192]
            xcm0, xcm1 = xin[:, 192:208], xin[:, 208:224]
            m256 = msk[:, 0:256]
            id128 = msk[:, 256:384]


            # ---------- cos(phi)/sin(phi) via Taylor ([128,1]) ----------
            t2 = cp.tile([128, 1], f32, tag="t2")
            nc.vector.tensor_tensor(t2[:], wq, wq, OP.mult)
            cphi = cp.tile([128, 1], f32, tag="cphi")
            nc.vector.tensor_scalar(cphi[:], t2[:], -1.0 / 720.0, 1.0 / 24.0,
                                    OP.mult, OP.add)
            nc.vector.scalar_tensor_tensor(cphi[:], cphi[:], 1.0, t2[:],
                                           OP.mult, OP.mult)
            nc.vector.tensor_scalar(cphi[:], cphi[:], 1.0, -0.5, OP.mult,
                                    OP.add)
            nc.vector.scalar_tensor_tensor(cphi[:], cphi[:], 1.0, t2[:],
                                           OP.mult, OP.mult)
            nc.vector.tensor_scalar(cphi[:], cphi[:], 1.0, 1.0, OP.mult,
                                    OP.add)
            sphi = cp.tile([128, 1], f32, tag="sphi")
            nc.vector.tensor_scalar(sphi[:], t2[:], -1.0 / 5040.0, 1.0 / 120.0,
                                    OP.mult, OP.add)
            nc.vector.scalar_tensor_tensor(sphi[:], sphi[:], 1.0, t2[:],
                                           OP.mult, OP.mult)
            nc.vector.tensor_scalar(sphi[:], sphi[:], 1.0, -1.0 / 6.0,
                                    OP.mult, OP.add)
            nc.vector.scalar_tensor_tensor(sphi[:], sphi[:], 1.0, t2[:],
                                           OP.mult, OP.mult)
            nc.vector.tensor_scalar(sphi[:], sphi[:], 1.0, 1.0, OP.mult,
                                    OP.add)
            nc.vector.scalar_tensor_tensor(sphi[:], sphi[:], 1.0, wq,
                                           OP.mult, OP.mult)
            # gate consts: ga2 = ga - gc/1000
            ga2 = cp.tile([128, 1], f32, tag="ga2")
            nc.vector.scalar_tensor_tensor(ga2[:], wgc, -1.0 / EMBED_CONST,
                                           wga, OP.mult, OP.add)

            epsb = cp.tile([128, 1], f32, tag="epsb")
            nc.vector.memset(epsb[:], EPS)

            def amps_batched(specs):
                # specs: list of (x0_ap, x1_ap, shape, tag, want_v)
                # batched so Ln and Exp activations group by function
                ms, lns, rs = [], [], []
                for x0, x1, sh, tag, _ in specs:
                    sq = wp.tile(sh, f32, tag=tag + "_sq")
                    nc.vector.tensor_tensor(sq[:], x0, x0, OP.mult)
                    sq1 = wp.tile(sh, f32, tag=tag + "_sq1")
                    nc.vector.tensor_tensor(sq1[:], x1, x1, OP.mult)
                    m = wp.tile(sh, f32, tag=tag + "_m")
                    nc.vector.tensor_tensor(m[:], sq[:], sq1[:], OP.add)
                    ms.append(m)
                for (x0, x1, sh, tag, _), m in zip(specs, ms):
                    ln = wp.tile(sh, f32, tag=tag + "_ln1")
                    nc.scalar.activation(ln[:], m[:], AF.Ln, bias=epsb[:],
                                         scale=0.5)
                    lns.append(ln)
                for (x0, x1, sh, tag, _), ln in zip(specs, lns):
                    r = wp.tile(sh, f32, tag=tag + "_rr1")
                    nc.scalar.activation(r[:], ln[:], AF.Exp, scale=-0.5)
                    rs.append(r)
                xn0s, ams, lns2, ras, vs = [], [], [], [], []
                for (x0, x1, sh, tag, want_v), r in zip(specs, rs):
                    xn0 = wp.tile(sh, f32, tag=tag + "_xn0")
                    nc.vector.tensor_tensor(xn0[:], x0, r[:], OP.mult)
                    v = None
                    if want_v:
                        xn1 = wp.tile(sh, f32, tag=tag + "_xn1")
                        nc.vector.tensor_tensor(xn1[:], x1, r[:], OP.mult)
                        v = wp.tile(sh, f32, tag=tag + "_v")
                        nc.vector.tensor_scalar(v[:], xn1[:], wv, None,
                                                OP.mult)
                    vs.append(v)
                    am = wp.tile(sh, f32, tag=tag + "_am")
                    nc.vector.tensor_tensor(am[:], xn0[:], xn0[:], OP.mult)
                    xn0s.append(xn0)
                    ams.append(am)
                for (x0, x1, sh, tag, _), am in zip(specs, ams):
                    ln = wp.tile(sh, f32, tag=tag + "_ln2")
                    nc.scalar.activation(ln[:], am[:], AF.Ln, bias=epsb[:],
                                         scale=0.5)
                    lns2.append(ln)
                for (x0, x1, sh, tag, _), ln in zip(specs, lns2):
                    ra = wp.tile(sh, f32, tag=tag + "_rr2")
                    nc.scalar.activation(ra[:], ln[:], AF.Exp, scale=-0.5)
                    ras.append(ra)
                outs = []
                for (x0, x1, sh, tag, want_v), xn0, ra, v in zip(
                        specs, xn0s, ras, vs):
                    a = wp.tile(sh, f32, tag=tag + "_a")
                    nc.vector.tensor_tensor(a[:], xn0[:], ra[:], OP.mult)
                    outs.append((a, v))
                return outs

            def hilo_pack(srct, pool, tag, pk, w, slots):
                # 3-way bf16 split written into packed-tile column slots.
                # slots: (h_slots, m_slots, l_slot) column indices, width w.
                hs, ms, l0 = slots
                h = pk[:, w * hs[0]:w * (hs[0] + 1)]
                nc.vector.tensor_copy(h, srct[:])
                r1 = pool.tile(srct.shape, f32, tag=tag + "_r1f")
                nc.vector.tensor_tensor(r1[:], srct[:], h, OP.subtract)
                m = pk[:, w * ms[0]:w * (ms[0] + 1)]
                nc.vector.tensor_copy(m, r1[:])
                r2 = pool.tile(srct.shape, f32, tag=tag + "_r2f")
                nc.vector.tensor_tensor(r2[:], r1[:], m, OP.subtract)
                lo = pk[:, w * l0:w * (l0 + 1)]
                nc.vector.tensor_copy(lo, r2[:])
                for s_ in hs[1:]:
                    nc.vector.tensor_copy(pk[:, w * s_:w * (s_ + 1)], h)
                for s_ in ms[1:]:
                    nc.vector.tensor_copy(pk[:, w * s_:w * (s_ + 1)], m)

            # ---------- batched prep: key (rm), query (own rm), bias (cm) --
            (ak, vk), (aq, _), (acm, _) = amps_batched([
                (xq0, xq1, [128, 32], "k", True),
                (xqo0, xqo1, [128, 16], "q", False),
                (xcm0, xcm1, [128, 16], "b", False),
            ])
            g1 = wp.tile([128, 32], f32, tag="g1")
            nc.vector.tensor_tensor(g1[:], ak[:], cth, OP.mult)
            g2 = wp.tile([128, 32], f32, tag="g2")
            nc.vector.tensor_tensor(g2[:], ak[:], sth, OP.mult)
            # packed KR rows: (g1h g1m g1l g1h g1m g1h | g2h g2m g2l ...)
            PK = cp.tile([128, 384], bf16, tag="PK")
            hilo_pack(g1, wp, "g1", PK, 32, ((0, 3, 5), (1, 4), 2))
            hilo_pack(g2, wp, "g2", PK, 32, ((6, 9, 11), (7, 10), 8))

            # V broadcast in two 2048-wide halves so early chunks unblock
            # sooner; all on the gpsimd queue
            Vbc0 = cp.tile([128, SEGW], f32, tag="Vbc0")
            nc.gpsimd.dma_start(Vbc0[0:1, :], vk[0:64, :])
            nc.gpsimd.partition_broadcast(Vbc0[:, :], Vbc0[0:1, :])
            Vbc1 = cp.tile([128, SEGW], f32, tag="Vbc1")
            nc.gpsimd.dma_start(Vbc1[0:1, :], vk[64:128, :])
            nc.gpsimd.partition_broadcast(Vbc1[:, :], Vbc1[0:1, :])
            Vh = (Vbc0, Vbc1)

            # KR in two key-halves so early chunks unblock before all row
            # DMAs land; one packed 12-row DMA per half, then cheap
            # block-copies replicate into PE row-groups 1-3
            KRa = cp.tile([128, SEGW], bf16, tag="KRa")
            KRb = cp.tile([128, SEGW], bf16, tag="KRb")
            nc.sync.dma_start(
                KRa[0:12, :],
                PK[0:64, :].rearrange("p (r c) -> r p c", r=12))
            for gi in range(1, 4):
                (nc.sync if gi == 2 else nc.scalar).dma_start(
                    KRa[32 * gi:32 * gi + 12, :], KRa[0:12, :])
            KRh = (KRa, KRb)

            # ---------- query-side rows (owned order, row-major) ----------
            kap = wp.tile([128, 16], f32, tag="kap")
            nc.vector.tensor_scalar(kap[:], aq[:], SCALE, None, OP.mult)
            # cq = cth*cphi + sth*sphi ; sq = sth*cphi - cth*sphi
            cq = wp.tile([128, 16], f32, tag="cq")
            nc.vector.tensor_scalar(cq[:], ctho, cphi[:], None, OP.mult)
            tmp = wp.tile([128, 16], f32, tag="tmpq")
            nc.vector.tensor_scalar(tmp[:], stho, sphi[:], None, OP.mult)
            nc.vector.tensor_tensor(cq[:], cq[:], tmp[:], OP.add)
            sq_ = wp.tile([128, 16], f32, tag="sq_")
            nc.vector.tensor_scalar(sq_[:], stho, cphi[:], None, OP.mult)
            nc.vector.tensor_scalar(tmp[:], ctho, sphi[:], None, OP.mult)
            nc.vector.tensor_tensor(sq_[:], sq_[:], tmp[:], OP.subtract)
            f1 = wp.tile([128, 16], f32, tag="f1")
            nc.vector.tensor_tensor(f1[:], kap[:], cq[:], OP.mult)
            f2 = wp.tile([128, 16], f32, tag="f2")
            nc.vector.tensor_tensor(f2[:], kap[:], sq_[:], OP.mult)
            # packed QL rows: (f1h f1h f1h f1m f1m f1l | f2h f2h f2h ...)
            PQ = cp.tile([128, 192], bf16, tag="PQ")
            hilo_pack(f1, wp, "f1", PQ, 16, ((0, 1, 2), (3, 4), 5))
            hilo_pack(f2, wp, "f2", PQ, 16, ((6, 7, 8), (9, 10), 11))
            QLa = cp.tile([128, 1024], bf16, tag="QLa")
            QLb = cp.tile([128, 1024], bf16, tag="QLb")
            nc.scalar.dma_start(
                QLa[0:12, :],
                PQ[0:64, :].rearrange("p (r c) -> r p c", r=12))
            for gi in range(1, 4):
                (nc.scalar if gi == 2 else nc.sync).dma_start(
                    QLa[32 * gi:32 * gi + 12, :], QLa[0:12, :])
            QLh = (QLa, QLb)
            # out channel 0 is exactly x0 (deferred: off the hot queues)
            nc.sync.dma_start(out_d[:, 0:16], xin[:, 192:208])

            # ---------- bias prep (owned, column-major) ----------
            nega = wp.tile([128, 16], f32, tag="nega")
            nc.vector.tensor_scalar(nega[:], acm[:], -1.0, None, OP.mult)
            absa = wp.tile([128, 16], f32, tag="absa")
            nc.vector.tensor_tensor(absa[:], acm[:], nega[:], OP.max)
            biasl = cp.tile([128, 16], f32, tag="biasl")
            nc.vector.tensor_scalar(biasl[:], absa[:], -SQ2 * SCALE, None,
                                    OP.mult)
            # epilogue precompute: hs0 = x0_owned^2
            hs0p = cp.tile([128, 16], f32, tag="hs0p")
            nc.vector.tensor_tensor(hs0p[:], xcm0, xcm0,
                                    OP.mult)

            # ---------- main loop ----------
            Dp = cp.tile([128, 32], f32, tag="Dp")
            nc.vector.memset(Dp[:], 0.0)
            Np = cp.tile([128, 32], f32, tag="Np")
            nc.vector.memset(Np[:], 0.0)
            outt2 = cp.tile([128, 16], f32, tag="outt2")

            def finalize(c0, c1):
                # combine + softmax + residual + gated MLP for chunk cols
                # [c0, c1); writes h1+df into outt2[:, c0:c1]
                hw_ = c1 - c0
                tg = f"_{c0}"

                def ft(tag, w=None):
                    return wp.tile([128, w or hw_], f32, tag=tag + tg,
                                   name=tag + tg)

                Dl = ft("Dl")
                nc.vector.tensor_tensor(Dl[:], Dp[:, 2 * c0:2 * c1:2],
                                        Dp[:, 2 * c0 + 1:2 * c1:2], OP.add)
                Nl = ft("Nl")
                nc.vector.tensor_tensor(Nl[:], Np[:, 2 * c0:2 * c1:2],
                                        Np[:, 2 * c0 + 1:2 * c1:2], OP.add)
                rD = ft("rD")
                nc.vector.reciprocal(rD[:], Dl[:])
                o0 = ft("o0")
                nc.vector.tensor_tensor(o0[:], Nl[:], rD[:], OP.mult)

                x0o = xin[:, 192 + c0:192 + c1]
                x1o = xin[:, 208 + c0:208 + c1]
                h1 = ft("h1")
                nc.vector.tensor_tensor(h1[:], x1o, o0[:], OP.add)
                hs1 = ft("hs1")
                nc.vector.tensor_tensor(hs1[:], h1[:], h1[:], OP.mult)
                mh = ft("mh")
                nc.vector.tensor_tensor(mh[:], hs0p[:, c0:c1], hs1[:], OP.add)
                lnh = ft("lnh")
                nc.scalar.activation(lnh[:], mh[:], AF.Ln, bias=epsb[:],
                                     scale=0.5)
                rh = ft("rh")
                nc.scalar.activation(rh[:], lnh[:], AF.Exp, scale=-0.5)
                hn0 = ft("hn0")
                nc.vector.tensor_tensor(hn0[:], x0o, rh[:], OP.mult)
                hn1 = ft("hn1")
                nc.vector.tensor_tensor(hn1[:], h1[:], rh[:], OP.mult)
                gt = ft("gt")
                nc.vector.tensor_scalar(gt[:], hn1[:], wgc, None, OP.mult)
                gpair = ft("gpair", 2 * hw_)
                nc.vector.scalar_tensor_tensor(gpair[:, 0:hw_], hn0[:],
                                               wga, gt[:], OP.mult, OP.add)
                nc.vector.scalar_tensor_tensor(gpair[:, hw_:2 * hw_], hn0[:],
                                               ga2[:], gt[:], OP.mult, OP.add)
                eg = ft("eg", 2 * hw_)
                nc.scalar.activation(eg[:], gpair[:], AF.Exp, scale=-1.0)
                nc.vector.tensor_scalar(eg[:], eg[:], 1.0, None, OP.add)
                rg = ft("rg", 2 * hw_)
                nc.vector.reciprocal(rg[:], eg[:])
                nc.vector.tensor_tensor(rg[:], gpair[:], rg[:], OP.mult)
                df = ft("df")
                nc.vector.tensor_tensor(df[:], rg[:, hw_:2 * hw_],
                                        rg[:, 0:hw_], OP.subtract)
                nc.vector.scalar_tensor_tensor(df[:], df[:], wc, hn0[:],
                                               OP.mult, OP.mult)
                nc.vector.tensor_tensor(outt2[:, c0:c1], h1[:], df[:], OP.add)

            def emit_late_rows():
                nc.sync.dma_start(
                    KRb[0:12, :],
                    PK[64:128, :].rearrange("p (r c) -> r p c", r=12))
                nc.gpsimd.dma_start(
                    QLb[0:12, :],
                    PQ[64:128, :].rearrange("p (r c) -> r p c", r=12))
                for gi in range(1, 4):
                    nc.sync.dma_start(KRb[32 * gi:32 * gi + 12, :],
                                      KRb[0:12, :])
                    nc.gpsimd.dma_start(QLb[32 * gi:32 * gi + 12, :],
                                        QLb[0:12, :])

            for lc in range(NCH):
                W = 256 * (lc + 1)
                nseg = (W + SEGW - 1) // SEGW
                for s in range(nseg):
                    j0 = s * SEGW
                    j1 = min(W, j0 + SEGW)
                    ww = j1 - j0
                    ps = pmm.tile([128, SEGW], f32)
                    last_seg = (j1 == W)
                    QLt = QLh[lc // 8]
                    q0 = 128 * (lc % 8)
                    for k, mo in enumerate(range(j0, j1, 512)):
                        me = min(j1, mo + 512)
                        gi = k % 4 if lc >= 2 else 0
                        nc.tensor.matmul(
                            ps[:, mo - j0:me - j0],
                            QLt[32 * gi:32 * gi + 12, q0:q0 + 128],
                            KRh[s][32 * gi:32 * gi + 12,
                                   mo - j0:me - j0],
                            start=True,
                            stop=not (last_seg and me == j1),
                            tile_position=(32 * gi, 0))
                    if last_seg:
                        nc.tensor.matmul(ps[:, W - 256 - j0:W - j0],
                                         id128, m256,
                                         start=False, stop=True)
                    e = ep.tile([128, SEGW], f32)
                    col = 2 * lc + s
                    nc.scalar.activation(e[:, 0:ww], ps[:, 0:ww], AF.Exp,
                                         bias=biasl[:, lc:lc + 1],
                                         accum_out=Dp[:, col:col + 1])
                    nc.vector.scalar_tensor_tensor(
                        e[:, 0:ww], e[:, 0:ww], 1.0, Vh[s][:, 0:ww],
                        OP.mult, OP.mult,
                        accum_out=Np[:, col:col + 1])
                if lc == 0:
                    emit_late_rows()
            finalize(0, 16)
            nc.sync.dma_start(out_d[:, 16:32], outt2[:])

    nc.compile()
    return nc


def _host_inputs(x, mask, q_weight, v_weight, gate_weight, carry_weight):
    """Per-core input maps. Host work is layout/indexing + constants only."""
    from ml_dtypes import bfloat16
    f32 = np.float32
    x = np.ascontiguousarray(x, dtype=f32)
    pos = np.arange(L)
    cth = np.cos(OMEGA * pos).astype(f32)
    sth = np.sin(OMEGA * pos).astype(f32)

    def rm(a0, a1, width):
        out = np.empty((128, 2 * width), f32)
        out[:, 0:width] = a0.reshape(128, width)
        out[:, width:2 * width] = a1.reshape(128, width)
        return out

    trig_rm = rm(cth, sth, 32)
    # causal masks for the last-256 window
    tri = np.where(np.arange(128)[None, :] <= np.arange(128)[:, None],
                   0.0, NEG).astype(f32)
    m_even = np.concatenate([tri, np.full((128, 128), NEG, f32)], axis=1)
    m_odd = np.concatenate([np.zeros((128, 128), f32), tri], axis=1)
    id128 = np.eye(128, dtype=f32)

    def rep(w):
        return np.full((128, 1), w, f32)

    in_maps = []
    for core in range(N_CORES):
        b, par = core // 2, core % 2
        xb = x[b]                              # [4096, 2]
        # owned positions in local order o: global = 256*(o//128)+128*par+(o%128)
        o = np.arange(2048)
        gown = 256 * (o // 128) + 128 * par + (o % 128)
        xin = np.concatenate([
            rm(xb[:, 0], xb[:, 1], 32),
            trig_rm,
            rm(xb[gown, 0], xb[gown, 1], 16),
            rm(cth[gown], sth[gown], 16),
            np.concatenate([xb[gown, 0].reshape(16, 128).T,
                            xb[gown, 1].reshape(16, 128).T], axis=1),
        ], axis=1).astype(f32)
        wts = np.zeros((128, 8), f32)
        wts[:, 0] = np.asarray(q_weight, f32)[0]
        wts[:, 1] = np.asarray(v_weight, f32)[0]
        wts[:, 2] = np.asarray(gate_weight, f32)[0]
        wts[:, 3] = np.asarray(gate_weight, f32)[1]
        wts[:, 4] = np.asarray(carry_weight, f32)[0]
        msk = np.concatenate(
            [(m_even if par == 0 else m_odd), id128], axis=1)
        in_maps.append({
            "xin": xin,
            "wts": wts,
            "msk": msk.astype(bfloat16),
        })
    return in_maps


def kernel(x, mask, q_weight, v_weight, gate_weight, carry_weight,
           _want_results=False):
    global _compiled
    from concourse.bass_utils import run_bass_kernel_spmd

    if _compiled is None:
        _compiled = _build()
    in_maps = _host_inputs(x, mask, q_weight, v_weight, gate_weight,
                           carry_weight)
    res = run_bass_kernel_spmd(_compiled, in_maps, list(range(N_CORES)))
    out = np.empty((B, L, 2), np.float32)
    for b in range(B):
        resh = out[b].reshape(16, 2, 128, 2)
        for par in range(2):
            r = res.results[2 * b + par]["out"]     # [128, 32]
            resh[:, par, :, 0] = r[:, 0:16].T
            resh[:, par, :, 1] = r[:, 16:32].T
    if _want_results:
        return out, res
    return out
